# revision 26
# baseline (speedup 1.0000x reference)
"""Reconstruction of the v1 kernel that passed on HW (for bisection)."""

from contextlib import ExitStack

import numpy as np

import concourse.bass as bass
import concourse.tile as tile
from concourse import bacc, mybir
from concourse.bass_utils import run_bass_kernel_spmd
from concourse.masks import make_identity

B = 16384
D = 128
NCLS = 100
NCORES = 8
NBLK = 13

F32 = mybir.dt.float32
I32 = mybir.dt.int32

_prog_cache = {}
TRACE = False
LAST_RESULTS = None


def _build(C, iters=1):
    R = NBLK * C
    CH = R // 128
    CPB = C // 128

    nc = bacc.Bacc("TRN2", target_bir_lowering=False, debug=False)
    xa = nc.dram_tensor("xa", [B, D], F32, kind="ExternalInput").ap()
    idx = nc.dram_tensor("idx", [128, CH], I32, kind="ExternalInput").ap()
    qrow = nc.dram_tensor("qrow", [1, R], F32, kind="ExternalInput").ap()
    pcol = nc.dram_tensor("pcol", [128, CH], F32, kind="ExternalInput").ap()
    out = nc.dram_tensor("out", [128, CH], F32, kind="ExternalOutput").ap()

    with ExitStack() as ctx:
        tc = ctx.enter_context(tile.TileContext(nc))
        const = ctx.enter_context(tc.tile_pool(name="const", bufs=1))
        natp = ctx.enter_context(tc.tile_pool(name="nat", bufs=4))
        sqscp = ctx.enter_context(tc.tile_pool(name="sqsc", bufs=2))
        sqbp = ctx.enter_context(tc.tile_pool(name="sqb", bufs=2))
        nbp = ctx.enter_context(tc.tile_pool(name="nb", bufs=3))
        t1p = ctx.enter_context(tc.tile_pool(name="t1", bufs=3))
        t2p = ctx.enter_context(tc.tile_pool(name="t2", bufs=2))
        pstp = ctx.enter_context(tc.tile_pool(name="pst", bufs=2, space="PSUM"))
        psgp = ctx.enter_context(tc.tile_pool(name="psg", bufs=3, space="PSUM"))
        psnp = ctx.enter_context(tc.tile_pool(name="psn", bufs=2, space="PSUM"))

        identity = const.tile([128, 128], F32)
        make_identity(nc, identity[:])
        notI = const.tile([128, 128], F32)
        nc.gpsimd.memset(notI[:], 1.0)
        nc.gpsimd.affine_select(
            out=notI[:],
            in_=notI[:],
            compare_op=mybir.AluOpType.not_equal,
            fill=0.0,
            base=0,
            pattern=[[-1, 128]],
            channel_multiplier=1,
        )
        ones_col = const.tile([128, 1], F32)
        nc.vector.memset(ones_col[:], 1.0)
        neghalf = const.tile([1, 128], F32)
        nc.vector.memset(neghalf[:], -0.5)

        idx_sb = const.tile([128, CH], I32)
        nc.sync.dma_start(out=idx_sb[:], in_=idx)
        qrow_sb = const.tile([1, R], F32)
        nc.sync.dma_start(out=qrow_sb[:], in_=qrow)
        pcol_sb = const.tile([128, CH], F32)
        nc.sync.dma_start(out=pcol_sb[:], in_=pcol)

        xgT = const.tile([128, R], F32)
        rs = const.tile([128, CH], F32)

        for b in [bb for _ in range(iters) for bb in range(NBLK)]:
            nb_col = nbp.tile([128, CPB], F32, tag="nb_col")
            for cc in range(CPB):
                c = b * CPB + cc
                nat = natp.tile([128, 128], F32)
                nc.gpsimd.indirect_dma_start(
                    out=nat[:],
                    out_offset=None,
                    in_=xa[:, :],
                    in_offset=bass.IndirectOffsetOnAxis(ap=idx_sb[:, c : c + 1], axis=0),
                )
                pst = pstp.tile([128, 128], F32)
                nc.tensor.transpose(out=pst[:], in_=nat[:], identity=identity[:])
                nc.vector.tensor_copy(out=xgT[:, c * 128 : (c + 1) * 128], in_=pst[:])
                sqsc = sqscp.tile([128, 128], F32)
                nc.scalar.activation(
                    out=sqsc[:],
                    in_=nat[:],
                    func=mybir.ActivationFunctionType.Square,
                    accum_out=nb_col[:, cc : cc + 1],
                )
            nb_aug = nbp.tile([128, CPB], F32, tag="nb_aug")
            nc.vector.tensor_add(
                out=nb_aug[:], in0=nb_col[:], in1=pcol_sb[:, b * CPB : (b + 1) * CPB]
            )
            xb = xgT[:, b * C : (b + 1) * C]
            sqb = sqbp.tile([128, C], F32)
            nc.vector.tensor_tensor(
                out=sqb[:], in0=xb, in1=xb, op=mybir.AluOpType.mult
            )
            psn = psnp.tile([1, C], F32, tag="psn")
            nc.tensor.matmul(out=psn[:], lhsT=ones_col[:], rhs=sqb[:], start=True, stop=True)
            nb_row = nbp.tile([1, C], F32, tag="nb_row")
            nc.vector.tensor_add(
                out=nb_row[:], in0=psn[:], in1=qrow_sb[:, b * C : (b + 1) * C]
            )
            for h in range(CPB):
                r = b * CPB + h
                psg = psgp.tile([128, C], F32)
                nc.tensor.matmul(
                    out=psg[:],
                    lhsT=xgT[:, r * 128 : (r + 1) * 128],
                    rhs=xb,
                    start=True,
                    stop=False,
                )
                nc.tensor.matmul(
                    out=psg[:], lhsT=neghalf[:], rhs=nb_row[:], start=False, stop=True
                )
                t1 = t1p.tile([128, C], F32)
                nc.scalar.activation(
                    out=t1[:],
                    in_=psg[:],
                    func=mybir.ActivationFunctionType.Relu,
                    bias=nb_aug[:, h : h + 1],
                    scale=-2.0,
                )
                nc.vector.tensor_tensor(
                    out=t1[:, h * 128 : (h + 1) * 128],
                    in0=t1[:, h * 128 : (h + 1) * 128],
                    in1=notI[:],
                    op=mybir.AluOpType.mult,
                )
                t2 = t2p.tile([128, C], F32)
                nc.scalar.activation(
                    out=t2[:],
                    in_=t1[:],
                    func=mybir.ActivationFunctionType.Sqrt,
                    accum_out=rs[:, r : r + 1],
                )

        nc.sync.dma_start(out=out[:, :], in_=rs[:])

    nc.compile()
    return nc


def _prep_inputs(x, target, C):
    R = NBLK * C
    CH = R // 128
    t = np.asarray(target).astype(np.int64).ravel()
    order = np.argsort(t, kind="stable").astype(np.int32)
    counts = np.bincount(t, minlength=NCORES * NBLK)
    starts = np.concatenate([[0], np.cumsum(counts)])

    xa = np.ascontiguousarray(np.asarray(x, dtype=np.float32))

    in_maps = []
    for core in range(NCORES):
        idx = np.zeros((R,), dtype=np.int32)  # pad -> row 0; penalties kill it
        pen = np.full((R,), -1e9, dtype=np.float32)
        for b in range(NBLK):
            k = core * NBLK + b
            cnt = int(counts[k]) if k < len(counts) else 0
            if cnt > 0:
                idx[b * C : b * C + cnt] = order[starts[k] : starts[k] + cnt]
                pen[b * C : b * C + cnt] = 0.0
        in_maps.append(
            {
                "xa": xa,
                "idx": np.ascontiguousarray(idx.reshape(CH, 128).T),
                "qrow": pen.reshape(1, R),
                "pcol": np.ascontiguousarray(pen.reshape(CH, 128).T),
            }
        )
    return in_maps


def kernel(x, target):
    t = np.asarray(target).astype(np.int64).ravel()
    counts = np.bincount(t, minlength=NCLS)
    C = max(256, ((int(counts.max()) + 127) // 128) * 128)
    if C not in _prog_cache:
        _prog_cache[C] = _build(C)
    nc = _prog_cache[C]
    in_maps = _prep_inputs(x, target, C)
    global LAST_RESULTS
    results = run_bass_kernel_spmd(nc, in_maps, list(range(NCORES)), trace=TRACE)
    LAST_RESULTS = results
    total = float(sum(np.asarray(r["out"], dtype=np.float64).sum() for r in results.results))
    return np.float32(total / 2.0 / B)
